# revision 23
# baseline (speedup 1.0000x reference)
# Trainium2 Bass kernel v3 for nn_MetricLearningLoss1 (triplet loss with
# semi-hard negative mining over top-k-confidence-filtered embeddings).
#
# Strategy (8 NeuronCores, SPMD, no collectives):
#   host: top-k filter, sort by label, per-core anchor row-blocks; every core
#         holds the full fp16 embedding matrix as matmul rhs.
#         The per-anchor threshold thr = (hardest-positive m-value) is
#         computed ON HOST by simulating device fp16 arithmetic over
#         same-label pairs (~85k dots), then shipped as 8 e4m3 levels in the
#         lhsT of a rank-12 fp8 DoubleRow matmul. The PE therefore leaves
#         y[a,j] = m[a,j] - thr_a - eps directly in PSUM (m = sqn_j - 2 a.e_j,
#         sqn via 4 e4m3 levels x 2.0 as in v2). Candidate columns (semi-hard
#         negatives) are exactly {y > 0}; positives are excluded because
#         y <= -eps + quantization-margin for them by construction.
#   device (per core), per 128-anchor tile, per column unit (6x1024 + 1x512):
#         'B' units: DVE u32 tensor_reduce min directly on the PSUM f32 bits
#                    (positive floats sort below negatives as unsigned ints)
#                    -> per-unit column mins, no drain at all.
#         'P' units: ScalarE copy PSUM->f16 x, then DVE u16 tensor_tensor
#                    mins into a [128,512] accumulator (Pool cannot do
#                    integer min: NCC_EBIR039).
#         No on-device finalize: the per-tile partial results (bpart u32,
#         acc u16) DMA out; the final min and the loss assembly are host-side.
#         The repeat loop unrolls UNROLL sweeps per For_i iteration: For_i
#         places an all-engine barrier + semaphore reset every iteration,
#         which flushes the pipeline; unrolling amortizes it.
#   host: decode partials -> hard_neg; exact hard_pos from fp32 embeddings;
#         rare rows with no semi-hard candidate recomputed exactly.
import sys

sys.path.insert(0, "/opt/trn_rl_repo")

from contextlib import ExitStack

import numpy as np

# ---------------------------------------------------------------- constants
N_FULL, D = 32768, 256
TOPK = int(0.2 * N_FULL)  # 6553
NCORES = 8
MARGIN = np.float32(0.075)
# The DR matmul uses zero-interleaved pairs, which makes the sqn+thr
# injection bit-exact on HW (the PE rounds each DR row-PAIR sum to
# ~fp16 precision; pairing every level with 0 avoids that).  The only
# remaining device-vs-host divergence is fp16-dot fp32 accumulation
# order (measured < 3e-4), so EPS only needs to cover that.
EPS = np.float32(2e-3)
# decode-side insurance: rows whose smallest candidate sits this close to
# the threshold are recomputed exactly on host
PATCH_BAND = np.float32(1e-4)

FULL_DIMS = dict(n=TOPK, npad=896, ntiles=7, W=256, blk=512)

# column units: (first 512-block, n blocks, mode). 'B' = PSUM u32 reduce on
# DVE; 'P' = ScalarE drain + DVE u16 min chain.
MODES = [
    (0, 2, "P"),
    (2, 2, "P"),
    (4, 2, "P"),
    (6, 2, "P"),
    (8, 2, "P"),
    (10, 2, "P"),
    (12, 1, "P"),
]
NLEV_SQ = 4  # e4m3 levels for sqn/2 (weight 2.0)
NLEV_TH = 8  # e4m3 levels for -(thr+eps) (weight 1.0)
# DR matmul partitions, zero-interleaved: each partition's row pair is
# (value, 0) so the PE's per-pair fp16-grade rounding never fires.
KDR = NLEV_SQ + NLEV_TH  # 12
UNROLL = 1  # sweeps per For_i iteration


def _nblocks(dims):
    return (dims["n"] + dims["blk"] - 1) // dims["blk"]


def _bcols(modes):
    # u32 partial-min output columns: one per B-unit 512-block reduce
    cols = 0
    for _, nb, mode in modes:
        if mode == "B":
            cols += nb
    return cols


# ---------------------------------------------------------------- builder
def build_nc(dims, repeat=1, modes=None, debug_x=False):
    import concourse.tile as tile
    from concourse import bacc, mybir

    if modes is None:
        modes = MODES
    n, npad, ntiles, blk = dims["n"], dims["npad"], dims["ntiles"], dims["blk"]
    NB = _nblocks(dims)
    NBC = _bcols(modes)
    assert npad == ntiles * 128

    nc = bacc.Bacc(
        "TRN2", target_bir_lowering=False, debug=False, num_devices=NCORES
    )
    f16, f32, u16 = mybir.dt.float16, mybir.dt.float32, mybir.dt.uint16
    u32 = mybir.dt.uint32
    f8 = mybir.dt.float8e4
    Alu = mybir.AluOpType
    X = mybir.AxisListType.X
    DR = mybir.MatmulPerfMode.DoubleRow

    L_d = nc.dram_tensor("L", [2, 128, npad], f16, kind="ExternalInput").ap()
    R_d = nc.dram_tensor("R", [NB, 128, 2 * blk], f16, kind="ExternalInput").ap()
    SQT_d = nc.dram_tensor("SQT", [KDR, 2, NB * blk], f8, kind="ExternalInput").ap()
    LT_d = nc.dram_tensor("LT", [KDR, 2, npad], f8, kind="ExternalInput").ap()
    ACC_d = nc.dram_tensor(
        "acc_out", [ntiles, 128, 512], u16, kind="ExternalOutput"
    ).ap()
    BP_d = None
    if NBC:
        BP_d = nc.dram_tensor(
            "bp_out", [ntiles, 128, NBC], u32, kind="ExternalOutput"
        ).ap()
    X_d = None
    if debug_x:
        X_d = nc.dram_tensor(
            "x_out", [ntiles, 2, 128, 1024], f16, kind="ExternalOutput"
        ).ap()

    with tile.TileContext(nc) as tc, ExitStack() as ctx:
        rpool = ctx.enter_context(tc.tile_pool(name="r", bufs=1))
        lpool = ctx.enter_context(tc.tile_pool(name="l", bufs=1))
        sqpool = ctx.enter_context(tc.tile_pool(name="sq", bufs=1))
        psum = ctx.enter_context(tc.tile_pool(name="ps", bufs=8, space="PSUM"))
        xpool = ctx.enter_context(tc.tile_pool(name="x", bufs=4))
        apool = ctx.enter_context(tc.tile_pool(name="acc", bufs=3))
        bpool = ctx.enter_context(tc.tile_pool(name="bp", bufs=3))

        lt = []
        for c in range(2):
            t_ = lpool.tile([128, npad], f16, tag=f"l{c}")
            lt.append(t_)
        sqt = sqpool.tile([KDR, 2, NB * blk], f8, tag="sqt")
        lt8 = sqpool.tile([KDR, 2, npad], f8, tag="lt8")
        rt = {}
        rtiles = []
        for b in range(NB):
            t_ = rpool.tile([128, 2 * blk], f16, tag=f"r{b}")
            rtiles.append(t_)
            rt[(0, b)] = t_[:, :blk]
            rt[(1, b)] = t_[:, blk:]

        # input DMAs (once, outside the repeat loop); minimal dep set first
        nc.sync.dma_start(out=lt[0][:, :128], in_=L_d[0][:, :128])
        nc.sync.dma_start(out=lt[1][:, :128], in_=L_d[1][:, :128])
        nc.sync.dma_start(out=lt8[:], in_=LT_d)
        nc.sync.dma_start(out=rtiles[0][:], in_=R_d[0])
        nc.sync.dma_start(out=sqt[:, :, : 2 * blk], in_=SQT_d[:, :, : 2 * blk])
        nc.sync.dma_start(out=rtiles[1][:], in_=R_d[1])
        nc.sync.dma_start(out=lt[0][:, 128:], in_=L_d[0][:, 128:])
        nc.sync.dma_start(out=lt[1][:, 128:], in_=L_d[1][:, 128:])
        for b in range(2, NB):
            nc.sync.dma_start(out=rtiles[b][:], in_=R_d[b])
            if b == 4:
                nc.sync.dma_start(
                    out=sqt[:, :, 2 * blk :], in_=SQT_d[:, :, 2 * blk :]
                )

        def sweep():
            for t in range(ntiles):
                tsl = slice(128 * t, 128 * (t + 1))
                bpart = None
                if NBC:
                    bpart = bpool.tile([128, NBC], u32, tag="bp")
                bcol = 0
                xs = []  # drained-but-unmerged x tiles: (tile, width)
                chain = []  # pairwise-merged [128,1024] u16 stages
                for b0, nb, mode in modes:
                    pts = []
                    for bi in range(nb):
                        b = b0 + bi
                        p = psum.tile([128, blk], f32, tag="pm")
                        pts.append(p)
                        nc.tensor.matmul(
                            p[:], lhsT=lt[0][:, tsl], rhs=rt[(0, b)],
                            start=True, stop=False,
                        )
                        nc.tensor.matmul(
                            p[:], lhsT=lt[1][:, tsl], rhs=rt[(1, b)],
                            start=False, stop=False,
                        )
                        nc.tensor.matmul(
                            p[:],
                            lhsT=lt8[:, :, tsl],
                            rhs=sqt[:, :, blk * b : blk * (b + 1)],
                            start=False,
                            stop=True,
                            perf_mode=DR,
                        )
                    if mode == "B":
                        for p in pts:
                            nc.vector.tensor_reduce(
                                out=bpart[:, bcol : bcol + 1],
                                in_=p[:].bitcast(u32),
                                axis=X,
                                op=Alu.min,
                            )
                            bcol += 1
                    else:
                        xb = xpool.tile([128, 1024], f16, tag="xb")
                        for bi, p in enumerate(pts):
                            nc.scalar.copy(
                                out=xb[:, blk * bi : blk * (bi + 1)], in_=p[:]
                            )
                        xs.append((xb, nb * blk))
                        if len(xs) == 2 and xs[0][1] == 1024 and xs[1][1] == 1024:
                            st = apool.tile([128, 1024], u16, tag=f"ch{len(chain)}")
                            nc.vector.tensor_tensor(
                                out=st[:],
                                in0=xs[0][0][:].bitcast(u16),
                                in1=xs[1][0][:].bitcast(u16),
                                op=Alu.min,
                            )
                            chain.append(st)
                            xs.clear()
                assert bcol == NBC
                # merge chain stages pairwise into chain[0]
                for st in chain[1:]:
                    nc.vector.tensor_tensor(
                        out=chain[0][:], in0=chain[0][:], in1=st[:], op=Alu.min
                    )
                acc = apool.tile([128, 512], u16, tag="acc")
                if chain:
                    nc.vector.tensor_tensor(
                        out=acc[:],
                        in0=chain[0][:, :512],
                        in1=chain[0][:, 512:],
                        op=Alu.min,
                    )
                    for xb, w in xs:  # leftover (single) units
                        for o in range(0, w, 512):
                            nc.vector.tensor_tensor(
                                out=acc[:],
                                in0=acc[:],
                                in1=xb[:, o : o + 512].bitcast(u16),
                                op=Alu.min,
                            )
                else:
                    nc.vector.memset(acc[:], float(0xFFFF))
                nc.sync.dma_start(out=ACC_d[t], in_=acc[:])
                if NBC:
                    nc.sync.dma_start(out=BP_d[t], in_=bpart[:, :NBC])

        if repeat == 1:
            sweep()
        else:
            full, rem = divmod(repeat, UNROLL)
            if full:
                with tc.For_i(0, full, 1):
                    for _ in range(UNROLL):
                        sweep()
            for _ in range(rem):
                sweep()

    nc.compile()
    return nc


_NC_CACHE = {}


def _get_nc(key, dims):
    if key not in _NC_CACHE:
        _NC_CACHE[key] = build_nc(dims)
    return _NC_CACHE[key]


# ---------------------------------------------------------------- host side
def _e4m3_levels(q, nlev):
    """Greedy e4m3 residual decomposition with saturation clipping."""
    from ml_dtypes import float8_e4m3 as npf8

    levels = []
    r = q.astype(np.float32)
    for _ in range(nlev):
        s = np.clip(r, -240.0, 240.0).astype(npf8)
        levels.append(s)
        r = r - s.astype(np.float32)
    return levels


def host_prep(embeddings, tags, confidences, dims, modes=None):
    from ml_dtypes import float8_e4m3 as npf8

    if modes is None:
        modes = MODES
    n, npad, ntiles, blk = dims["n"], dims["npad"], dims["ntiles"], dims["blk"]
    NB = _nblocks(dims)
    ncols = NB * blk
    conf = np.asarray(confidences, dtype=np.float32)
    order = np.argsort(-conf, kind="stable")[:n]
    emb = np.asarray(embeddings, dtype=np.float32)[order]
    labs = np.asarray(tags)[order]
    perm = np.argsort(labs, kind="stable")
    emb_s = np.ascontiguousarray(emb[perm], dtype=np.float32)
    labs_s = labs[perm]
    sqn = (emb_s**2).sum(axis=1, dtype=np.float32).astype(np.float32)
    counts = np.bincount(labs_s)
    valid = (counts[labs_s] >= 2) & (counts[labs_s] < n)

    # fp16 operands exactly as shipped to the device
    E16 = emb_s.astype(np.float16)  # R columns: f16(emb)
    L16 = (-2.0 * emb_s).astype(np.float16)  # lhsT columns: f16(-2 emb)
    EhiT = np.ascontiguousarray(E16.T)  # [256, n]

    # sqn levels (4 x e4m3 of sqn/2, weight 2.0), as in v2
    sq_levels = _e4m3_levels(sqn * np.float32(0.5), NLEV_SQ)
    sq32dev = np.float32(2.0) * sum(
        lv.astype(np.float32) for lv in sq_levels
    )

    # device-arithmetic hard-positive threshold + exact hard_pos, per label
    L32 = L16.astype(np.float32)
    E32 = E16.astype(np.float32)
    thr_raw = np.empty(n, np.float32)
    hp_exact = np.zeros(n, np.float32)
    seg_starts = np.searchsorted(labs_s, np.unique(labs_s), side="left")
    seg_ends = np.searchsorted(labs_s, np.unique(labs_s), side="right")
    for s0, s1 in zip(seg_starts, seg_ends):
        k = s1 - s0
        if k == 1:
            # single-member label: thr = device m(self) = -sqn-ish; anchor is
            # invalid anyway, this value just keeps everything finite
            thr_raw[s0] = L32[s0] @ E32[s0] + sq32dev[s0]
            continue
        Mdev = L32[s0:s1] @ E32[s0:s1].T + sq32dev[s0:s1][None, :]
        np.fill_diagonal(Mdev, -np.inf)
        thr_raw[s0:s1] = Mdev.max(axis=1)
        G = emb_s[s0:s1]
        D2 = (
            sqn[s0:s1][:, None]
            + sqn[s0:s1][None, :]
            - 2.0 * (G @ G.T).astype(np.float32)
        )
        np.fill_diagonal(D2, -np.inf)
        hp_exact[s0:s1] = np.sqrt(np.maximum(D2.max(axis=1), 0.0))

    # -(thr + eps) as 8 e4m3 levels (weight 1.0); exact device sum for decode
    q = -(thr_raw + EPS)
    th_levels_all = _e4m3_levels(q, NLEV_TH)
    nthr_dev = sum(lv.astype(np.float32) for lv in th_levels_all)

    # shared R and SQT (identical on every core)
    Rr = np.empty((2, 128, ncols), np.float16)
    Rr[0, :, :n] = EhiT[0:128]
    Rr[1, :, :n] = EhiT[128:256]
    Rr[:, :, n:] = 0
    R = np.ascontiguousarray(
        Rr.reshape(2, 128, NB, blk).transpose(2, 1, 0, 3).reshape(NB, 128, 2 * blk)
    )
    # zero-interleaved DR layout: partition kk carries (value-row, zero-row)
    SQT = np.zeros((KDR, 2, ncols), npf8)
    for li in range(NLEV_SQ):
        SQT[li, 0, :n] = sq_levels[li]
        SQT[li, 0, n:] = npf8(-240.0)  # pad cols: m ~ -1920, never a candidate
    SQT[NLEV_SQ:, 0, :] = npf8(1.0)  # thr rows: rhs = 1

    starts = [round(k * n / NCORES) for k in range(NCORES + 1)]
    cores, in_maps = [], []
    for k in range(NCORES):
        a0, a1 = starts[k], starts[k + 1]
        cnt = a1 - a0

        b = np.zeros((npad, D), np.float32)
        b[:cnt] = -2.0 * emb_s[a0:a1]
        bhiT = b.T.astype(np.float16)  # [256, npad]
        L = np.stack([bhiT[0:128], bhiT[128:256]])

        LT = np.zeros((KDR, 2, npad), npf8)
        LT[0:NLEV_SQ, 0, :] = npf8(2.0)
        qc = np.full(npad, -4000.0, np.float32)  # pad rows: y < 0 everywhere
        qc[:cnt] = q[a0:a1]
        lv = _e4m3_levels(qc, NLEV_TH)
        for li in range(NLEV_TH):
            LT[NLEV_SQ + li, 0, :] = lv[li]

        cores.append(dict(a0=a0, cnt=cnt))
        in_maps.append({"L": L, "R": R, "SQT": SQT, "LT": LT})
    return (
        dict(
            emb_s=emb_s,
            labs_s=labs_s,
            sqn=sqn,
            valid=valid,
            cores=cores,
            n=n,
            hp=hp_exact,
            nthr_dev=nthr_dev,
            modes=modes,
        ),
        in_maps,
    )


def host_decode(prep, outs):
    n = prep["n"]
    emb_s, labs_s, sqn, valid, hp = (
        prep["emb_s"],
        prep["labs_s"],
        prep["sqn"],
        prep["valid"],
        prep["hp"],
    )
    nthr_dev = prep["nthr_dev"]
    terms = np.zeros(n, np.float32)
    patch_rows = []
    inf = np.float32(np.inf)
    for k, core in enumerate(prep["cores"]):
        acc_v, bp_v = outs[k]  # [ntiles,128,512] u16, [ntiles,128,NBC] u32
        a0, cnt = core["a0"], core["cnt"]
        av = acc_v.reshape(-1, 512)[:cnt].view(np.float16).astype(np.float32)
        ystar = np.where(av > 0, av, inf).min(axis=1)
        if bp_v.size:
            bv = bp_v.reshape(-1, bp_v.shape[-1])[:cnt].view(np.float32)
            ystar = np.minimum(ystar, np.where(bv > 0, bv, inf).min(axis=1))
        g = a0 + np.arange(cnt)
        vmask = valid[g]
        suspicious = ~np.isfinite(ystar) | (ystar < PATCH_BAND)
        for i in np.nonzero(vmask & suspicious)[0]:
            patch_rows.append(a0 + int(i))
        ok = vmask & ~suspicious
        idx = np.nonzero(ok)[0]
        if idx.size == 0:
            continue
        gg = g[idx]
        mstar = ystar[idx] - nthr_dev[gg]  # y - sum(levels) = m - (thr+eps-dq)
        hn = np.sqrt(np.maximum(mstar + sqn[gg], 0.0), dtype=np.float32)
        terms[gg] = np.maximum(hp[gg] - hn + MARGIN, np.float32(0.0))

    patch_rows = sorted(set(patch_rows))
    if patch_rows:
        rows = np.array(patch_rows, np.int64)
        sq_rows = (
            sqn[rows][:, None]
            + sqn[None, :]
            - 2.0 * (emb_s[rows] @ emb_s.T).astype(np.float32)
        ).astype(np.float32)
        dist = np.sqrt(np.maximum(sq_rows, 0.0), dtype=np.float32)
        for ridx, gi in enumerate(rows):
            same = labs_s == labs_s[gi]
            pos = same.copy()
            pos[gi] = False
            neg = ~same
            if not pos.any() or not neg.any():
                terms[gi] = 0.0
                continue
            drow = dist[ridx]
            hard_pos = drow[pos].max()
            neg_min = drow[neg].min()
            shn = drow[neg & (drow > hard_pos)]
            hard_neg = shn.min() if shn.size else neg_min
            terms[gi] = max(hard_pos - hard_neg + MARGIN, np.float32(0.0))

    cnt_valid = valid.sum()
    if cnt_valid > 0:
        return np.float32(terms.sum(dtype=np.float32) / max(cnt_valid, 1))
    return np.float32(0.0)


# ---------------------------------------------------------------- entry
def kernel(embeddings, tags, confidences):
    from concourse.bass_utils import run_bass_kernel_spmd

    dims = FULL_DIMS
    nc = _get_nc("full", dims)
    prep, in_maps = host_prep(embeddings, tags, confidences, dims)
    res = run_bass_kernel_spmd(nc, in_maps, list(range(NCORES)))
    nbc = _bcols(MODES)
    outs = [
        (
            np.ascontiguousarray(res.results[k]["acc_out"]).astype(np.uint16),
            np.ascontiguousarray(res.results[k]["bp_out"]).astype(np.uint32)
            if nbc
            else np.zeros((FULL_DIMS["ntiles"], 128, 0), np.uint32),
        )
        for k in range(NCORES)
    ]
    loss = host_decode(prep, outs)
    return np.array(loss, dtype=np.float32)
